# revision 10
# baseline (speedup 1.0000x reference)
import sys
import numpy as np

sys.path.insert(0, "/opt/trn_rl_repo")

B, N, M = 8, 2048, 256
NCORES = 8
U = 64  # unroll factor inside hardware loops

_cache = {}


def _build():
    if "nc" in _cache:
        return _cache["nc"]
    from concourse import bass, tile, bacc
    import concourse.mybir as mybir

    fp32 = mybir.dt.float32
    Alu = mybir.AluOpType
    Act = mybir.ActivationFunctionType
    AX = mybir.AxisListType
    ds = bass.ds

    nc = bacc.Bacc("TRN2", target_bir_lowering=False, debug=False,
                   num_devices=NCORES)

    node_d = nc.dram_tensor("node", [N, M], fp32, kind="ExternalInput").ap()
    c0_d = nc.dram_tensor("c0", [128, 2], fp32, kind="ExternalInput").ap()
    h0_d = nc.dram_tensor("h0", [128, 2], fp32, kind="ExternalInput").ap()
    xb_d = nc.dram_tensor("xb", [128, 8], fp32, kind="ExternalInput").ap()
    wstat_d = nc.dram_tensor("wstat", [128, 2048], fp32, kind="ExternalInput").ap()
    wkt_d = nc.dram_tensor("wkt", [128, 512], fp32, kind="ExternalInput").ap()
    wqt_d = nc.dram_tensor("wqt", [128, 512], fp32, kind="ExternalInput").ap()
    bkt_d = nc.dram_tensor("bkt", [128, 2], fp32, kind="ExternalInput").ap()
    bqt_d = nc.dram_tensor("bqt", [128, 2], fp32, kind="ExternalInput").ap()
    ones_d = nc.dram_tensor("ones1", [1, 128], fp32, kind="ExternalInput").ap()
    id_d = nc.dram_tensor("ident", [128, 128], fp32, kind="ExternalInput").ap()
    tvs_d = nc.dram_tensor("tvs", [128, 16], fp32, kind="ExternalInput").ap()
    out_d = nc.dram_tensor("out", [N, N], fp32, kind="ExternalOutput").ap()

    with tile.TileContext(nc) as tc:
        # ---- constants in SBUF ----
        xb_sb, f_xb = tc.tile([128, 8], fp32, name="xb_sb")
        wstat_sb, f_wstat = tc.tile([128, 2048], fp32, name="wstat_sb")
        wkt_sb, f_wkt = tc.tile([128, 512], fp32, name="wkt_sb")
        wqt_sb, f_wqt = tc.tile([128, 512], fp32, name="wqt_sb")
        bkt_sb, f_bkt = tc.tile([128, 2], fp32, name="bkt_sb")
        bqt_sb, f_bqt = tc.tile([128, 2], fp32, name="bqt_sb")
        ones_sb, f_ones = tc.tile([1, 128], fp32, name="ones_sb")
        id_sb, f_id = tc.tile([128, 128], fp32, name="id_sb")
        tvs_sb, f_tvs = tc.tile([128, 16], fp32, name="tvs_sb")
        for sb, dr in ((xb_sb, xb_d), (wstat_sb, wstat_d), (wkt_sb, wkt_d),
                       (wqt_sb, wqt_d), (bkt_sb, bkt_d), (bqt_sb, bqt_d),
                       (ones_sb, ones_d), (id_sb, id_d), (tvs_sb, tvs_d)):
            nc.gpsimd.dma_start(sb[:], dr[:, :])

        # main PSUM pool used across all phases
        p512_cm = tc.tile_pool(name="p512", bufs=2, space="PSUM")
        p512 = p512_cm.__enter__()

        # ---- persistent big tensors ----
        keysT, f_keysT = tc.tile([128, 2, N], fp32, name="keysT")
        QT, f_QT = tc.tile([128, 2, N], fp32, name="QT")

        # ---- prologue: node -> nodeT -> keysT ----
        nodeN, f_nodeN = tc.tile([128, 16, 256], fp32, name="nodeN")
        nodeT, f_nodeT = tc.tile([128, 2, N], fp32, name="nodeT")
        for c in range(16):
            nc.gpsimd.dma_start(nodeN[:, c, :], node_d[c * 128:(c + 1) * 128, :])
        ptp_cm = tc.tile_pool(name="ptp", bufs=2, space="PSUM")
        ptp = ptp_cm.__enter__()
        for c in range(16):
            for k in range(2):
                pt = ptp.tile([128, 128], fp32)
                nc.tensor.transpose(pt[:], nodeN[:, c, k * 128:(k + 1) * 128], id_sb[:])
                nc.scalar.activation(nodeT[:, k, c * 128:(c + 1) * 128], pt[:], Act.Copy)
        for j2 in range(2):
            for nb in range(4):
                pk = p512.tile([128, 512], fp32, tag="ps")
                for k in range(2):
                    nc.tensor.matmul(pk[:], wkt_sb[:, (k * 2 + j2) * 128:(k * 2 + j2 + 1) * 128],
                                     nodeT[:, k, nb * 512:(nb + 1) * 512],
                                     start=(k == 0), stop=(k == 1))
                nc.vector.tensor_scalar(out=keysT[:, j2, nb * 512:(nb + 1) * 512],
                                        in0=pk[:], scalar1=bkt_sb[:, j2:j2 + 1],
                                        scalar2=None, op0=Alu.add)
        ptp_cm.__exit__(None, None, None)
        f_nodeT()
        f_nodeN()

        # ---- phase 1: LSTM unroll -> HTx ----
        HTx, f_HTx = tc.tile([128, 2, N + 1], fp32, name="HTx")
        ct, f_ct = tc.tile([128, 2], fp32, name="ct")
        gsb, f_gsb = tc.tile([128, 8], fp32, name="gsb")
        sfo, f_sfo = tc.tile([128, 6], fp32, name="sfo")
        gt, f_gt = tc.tile([128, 2], fp32, name="gt")
        t1, f_t1 = tc.tile([128, 2], fp32, name="t1")
        tct, f_tct = tc.tile([128, 2], fp32, name="tct")
        nc.gpsimd.dma_start(HTx[:, 0, 0:1], h0_d[:, 0:1])
        nc.gpsimd.dma_start(HTx[:, 1, 0:1], h0_d[:, 1:2])
        nc.gpsimd.dma_start(ct[:], c0_d[:, :])
        gpp_cm = tc.tile_pool(name="gpp", bufs=2, space="PSUM")
        gpp = gpp_cm.__enter__()
        with tc.For_i(0, N, step=U) as iv:
            for u in range(U):
                t = iv + u
                gp = gpp.tile([128, 8], fp32)
                for m2 in range(8):
                    nc.tensor.matmul(gp[:, m2:m2 + 1],
                                     wstat_sb[:, (m2 * 2) * 128:(m2 * 2 + 1) * 128],
                                     HTx[:, 0, ds(t, 1)], start=True, stop=False)
                    nc.tensor.matmul(gp[:, m2:m2 + 1],
                                     wstat_sb[:, (m2 * 2 + 1) * 128:(m2 * 2 + 2) * 128],
                                     HTx[:, 1, ds(t, 1)], start=False, stop=True)
                nc.vector.tensor_tensor(out=gsb[:], in0=gp[:], in1=xb_sb[:], op=Alu.add)
                nc.scalar.activation(sfo[:], gsb[:, 0:6], Act.Sigmoid)
                nc.scalar.activation(gt[:], gsb[:, 6:8], Act.Tanh)
                nc.vector.tensor_tensor(out=t1[:], in0=sfo[:, 0:2], in1=gt[:], op=Alu.mult)
                nc.vector.tensor_tensor(out=ct[:], in0=sfo[:, 2:4], in1=ct[:], op=Alu.mult)
                nc.vector.tensor_tensor(out=ct[:], in0=ct[:], in1=t1[:], op=Alu.add)
                nc.scalar.activation(tct[:], ct[:], Act.Tanh)
                nc.vector.tensor_tensor(out=HTx[:, 0, ds(t + 1, 1)],
                                        in0=sfo[:, 4:5], in1=tct[:, 0:1], op=Alu.mult)
                nc.vector.tensor_tensor(out=HTx[:, 1, ds(t + 1, 1)],
                                        in0=sfo[:, 5:6], in1=tct[:, 1:2], op=Alu.mult)
        gpp_cm.__exit__(None, None, None)

        # ---- QT = Wq @ h + bq (feature-on-partition) ----
        for j2 in range(2):
            for tb in range(4):
                pq = p512.tile([128, 512], fp32, tag="ps")
                for k in range(2):
                    nc.tensor.matmul(pq[:], wqt_sb[:, (k * 2 + j2) * 128:(k * 2 + j2 + 1) * 128],
                                     HTx[:, k, 1 + tb * 512:1 + (tb + 1) * 512],
                                     start=(k == 0), stop=(k == 1))
                nc.vector.tensor_scalar(out=QT[:, j2, tb * 512:(tb + 1) * 512],
                                        in0=pq[:], scalar1=bqt_sb[:, j2:j2 + 1],
                                        scalar2=None, op0=Alu.add)
        f_tct(); f_t1(); f_gt(); f_sfo(); f_gsb(); f_ct(); f_HTx()

        # ---- phase 2+3 interleaved: ST blocks + argmax-rank chain ----
        rb, f_rb = tc.tile([128, N], fp32, name="rb")
        trs, f_trs = tc.tile([16, 128], fp32, name="trs")
        rankn, f_rankn = tc.tile([128, 16], fp32, name="rankn")
        rr, f_rr = tc.tile([1, N], fp32, name="rr")
        stp_cm = tc.tile_pool(name="stp", bufs=2)
        stp = stp_cm.__enter__()
        ma, f_ma = tc.tile([128, 16], fp32, name="ma")
        ms, f_ms = tc.tile([128, 16], fp32, name="ms")
        mk, f_mk = tc.tile([128, 16], fp32, name="mk")
        pm, f_pm = tc.tile([128, 1], fp32, name="pm")
        gm, f_gm = tc.tile([1, 1], fp32, name="gm")
        dl, f_dl = tc.tile([128, 16], fp32, name="dl")
        tpp_cm = tc.tile_pool(name="tpp", bufs=2, space="PSUM")
        tpp = tpp_cm.__enter__()
        gbp_cm = tc.tile_pool(name="gbp", bufs=2, space="PSUM")
        gbp = gbp_cm.__enter__()
        nc.vector.memset(ma[:], 0.0)
        nc.vector.memset(ms[:], 0.0)

        def emit_st_block(tb):
            st_tb = stp.tile([128, 16, 512], fp32, name=f"st{tb}", tag="st")
            for c in range(16):
                pS = p512.tile([128, 512], fp32, tag="ps")
                for k in range(2):
                    nc.tensor.matmul(pS[:], keysT[:, k, c * 128:(c + 1) * 128],
                                     QT[:, k, tb * 512:(tb + 1) * 512],
                                     start=(k == 0), stop=(k == 1))
                nc.scalar.activation(st_tb[:, c, :], pS[:], Act.Copy)
            return st_tb

        def emit_l3(st_tb):
            with tc.For_i(0, 512, step=U) as iv:
                for u in range(U):
                    tl_ = iv + u
                    nc.vector.tensor_tensor(out=mk[:], in0=st_tb[:, :, ds(tl_, 1)],
                                            in1=ma[:], op=Alu.add)
                    nc.vector.tensor_tensor(out=ms[:], in0=ms[:], in1=ma[:], op=Alu.add)
                    nc.vector.reduce_max(out=pm[:], in_=mk[:], axis=AX.X)
                    tp = tpp.tile([1, 128], fp32, tag="tp")
                    nc.tensor.transpose(tp[:], pm[:], id_sb[:])
                    nc.vector.reduce_max(out=gm[:], in_=tp[:], axis=AX.X)
                    gb = gbp.tile([128, 1], fp32)
                    nc.tensor.matmul(gb[:], ones_sb[:], gm[:], start=True, stop=True)
                    nc.vector.tensor_scalar(out=dl[:], in0=mk[:], scalar1=gb[:],
                                            scalar2=-1e9, op0=Alu.is_ge, op1=Alu.mult)
                    nc.vector.tensor_tensor(out=ma[:], in0=ma[:], in1=dl[:], op=Alu.add)

        blocks = [emit_st_block(0), emit_st_block(1)]
        emit_l3(blocks[0])
        blocks.append(emit_st_block(2))
        emit_l3(blocks[1])
        blocks.append(emit_st_block(3))
        emit_l3(blocks[2])
        emit_l3(blocks[3])

        # rank_n = 2047 + msum/1e9 ; broadcast across partitions -> rb
        nc.vector.tensor_scalar(out=rankn[:], in0=ms[:], scalar1=1e-9,
                                scalar2=2047.0, op0=Alu.mult, op1=Alu.add)
        tp2 = tpp.tile([16, 128], fp32, tag="tp")
        nc.tensor.transpose(tp2[:], rankn[:], id_sb[:])
        nc.scalar.activation(trs[:], tp2[:], Act.Copy)
        for c in range(16):
            nc.gpsimd.dma_start(rr[0:1, c * 128:(c + 1) * 128], trs[c:c + 1, :])
        for g in range(4):
            pr = p512.tile([128, 512], fp32, tag="ps")
            nc.tensor.matmul(pr[:], ones_sb[:], rr[0:1, g * 512:(g + 1) * 512],
                             start=True, stop=True)
            nc.scalar.activation(rb[:, g * 512:(g + 1) * 512], pr[:], Act.Copy)
        gbp_cm.__exit__(None, None, None)
        tpp_cm.__exit__(None, None, None)
        f_dl(); f_gm(); f_pm(); f_mk(); f_ms(); f_ma()
        stp_cm.__exit__(None, None, None)

        # ---- phase 4: probs rows, masked softmax, DMA out ----
        rs4, f_rs4 = tc.tile([128, 4], fp32, name="rs4")
        rsum, f_rsum = tc.tile([128, 1], fp32, name="rsum")
        rinv, f_rinv = tc.tile([128, 1], fp32, name="rinv")
        esp_cm = tc.tile_pool(name="esp", bufs=2)
        esp = esp_cm.__enter__()
        eop_cm = tc.tile_pool(name="eop", bufs=2)
        eop = eop_cm.__enter__()
        for blk in range(16):
            es = esp.tile([128, N], fp32, name=f"es{blk}", tag="es")
            for nb in range(4):
                pS = p512.tile([128, 512], fp32, tag="ps")
                for k in range(2):
                    nc.tensor.matmul(pS[:], QT[:, k, blk * 128:(blk + 1) * 128],
                                     keysT[:, k, nb * 512:(nb + 1) * 512],
                                     start=(k == 0), stop=(k == 1))
                nc.scalar.activation(es[:, nb * 512:(nb + 1) * 512], pS[:], Act.Exp)
                nc.vector.scalar_tensor_tensor(
                    out=es[:, nb * 512:(nb + 1) * 512],
                    in0=rb[:, nb * 512:(nb + 1) * 512],
                    scalar=tvs_sb[:, blk:blk + 1],
                    in1=es[:, nb * 512:(nb + 1) * 512],
                    op0=Alu.is_ge, op1=Alu.mult,
                    accum_out=rs4[:, nb:nb + 1])
            nc.vector.reduce_sum(out=rsum[:], in_=rs4[:], axis=AX.X)
            nc.vector.reciprocal(rinv[:], rsum[:])
            eo = eop.tile([128, N], fp32, name=f"eo{blk}", tag="eo")
            for nb in range(4):
                nc.scalar.activation(eo[:, nb * 512:(nb + 1) * 512],
                                     es[:, nb * 512:(nb + 1) * 512],
                                     Act.Copy, scale=rinv[:])
            nc.gpsimd.dma_start(out_d[blk * 128:(blk + 1) * 128, :], eo[:])
        eop_cm.__exit__(None, None, None)
        esp_cm.__exit__(None, None, None)
        f_rinv(); f_rsum(); f_rs4()
        f_rr(); f_rankn(); f_trs(); f_rb()
        f_QT(); f_keysT()
        p512_cm.__exit__(None, None, None)
        f_tvs(); f_id(); f_ones(); f_bqt(); f_bkt(); f_wqt(); f_wkt(); f_wstat(); f_xb()

    nc.compile()
    _cache["nc"] = nc
    return nc


def _prep_inputs(inputs):
    f32 = np.float32
    node_embedding = np.ascontiguousarray(inputs["node_embedding"], dtype=f32)
    z_g = np.asarray(inputs["z_g"], dtype=f32)
    decoder_init = np.asarray(inputs["decoder_init"], dtype=f32)
    hidden0 = np.asarray(inputs["hidden0"], dtype=f32)
    w_ih = np.asarray(inputs["w_ih"], dtype=f32)
    w_hh = np.asarray(inputs["w_hh"], dtype=f32)
    b_ih = np.asarray(inputs["b_ih"], dtype=f32)
    b_hh = np.asarray(inputs["b_hh"], dtype=f32)
    Wq = np.asarray(inputs["Wq"], dtype=f32)
    bq = np.asarray(inputs["bq"], dtype=f32)
    Wk = np.asarray(inputs["Wk"], dtype=f32)
    bk = np.asarray(inputs["bk"], dtype=f32)

    perm = np.concatenate([np.arange(0, 256), np.arange(256, 512),
                           np.arange(768, 1024), np.arange(512, 768)])
    w_hh_p = w_hh[perm]
    x_proj = decoder_init @ w_ih.T + b_ih
    xb = np.ascontiguousarray(((x_proj + b_hh)[perm]).reshape(8, 128).T, dtype=f32)
    wstat = np.zeros((128, 2048), f32)
    for m2 in range(8):
        for k in range(2):
            blockT = w_hh_p[m2 * 128:(m2 + 1) * 128, k * 128:(k + 1) * 128].T
            wstat[:, (m2 * 2 + k) * 128:(m2 * 2 + k + 1) * 128] = blockT
    WkT = Wk.T
    WqT = Wq.T
    wkt = np.zeros((128, 512), f32)
    wqt = np.zeros((128, 512), f32)
    for k in range(2):
        for j in range(2):
            wkt[:, (k * 2 + j) * 128:(k * 2 + j + 1) * 128] = \
                WkT[k * 128:(k + 1) * 128, j * 128:(j + 1) * 128]
            wqt[:, (k * 2 + j) * 128:(k * 2 + j + 1) * 128] = \
                WqT[k * 128:(k + 1) * 128, j * 128:(j + 1) * 128]
    bkt = np.ascontiguousarray(bk.reshape(2, 128).T, dtype=f32)
    bqt = np.ascontiguousarray(bq.reshape(2, 128).T, dtype=f32)
    h0c = np.ascontiguousarray(hidden0.reshape(2, 128).T, dtype=f32)
    ones1 = np.ones((1, 128), f32)
    ident = np.eye(128, dtype=f32)
    tvs = (np.arange(128, dtype=f32)[:, None] +
           128.0 * np.arange(16, dtype=f32)[None, :]).astype(f32)

    shared = dict(xb=xb, wstat=wstat, wkt=wkt, wqt=wqt, bkt=bkt, bqt=bqt,
                  ones1=ones1, ident=ident, tvs=tvs, h0=h0c)
    in_maps = []
    for b in range(B):
        m = dict(shared)
        m["node"] = np.ascontiguousarray(node_embedding[b])
        m["c0"] = np.ascontiguousarray(z_g[b].reshape(2, 128).T, dtype=f32)
        in_maps.append(m)
    return in_maps


def _run(inputs, trace=False, tmpdir=None):
    nc = _build()
    in_maps = _prep_inputs(inputs)
    from concourse import bass_utils
    res = bass_utils.run_bass_kernel_spmd(nc, in_maps, core_ids=list(range(NCORES)),
                                          tmpdir=tmpdir, trace=trace)
    out = np.stack([np.asarray(r["out"], dtype=np.float32) for r in res.results], axis=0)
    return out, res


def kernel(**inputs) -> np.ndarray:
    out, _ = _run(inputs, trace=False)
    return out
